# revision 9
# baseline (speedup 1.0000x reference)
"""Trainium2 Bass kernel for nn_LookupTableLayer (embedding_lookup).

Full-input contract: kernel(**inputs) takes the full unsharded numpy inputs,
shards positions across 8 NeuronCores (batch dim), runs one SPMD NEFF on
cores 0-7, and returns the full [16, 512, 32, 128] f32 output.

Algorithm:
  out[..., 0:64] = pairsum(tx')[idx0], out[..., 64:128] = pairsum(ty')[idx1]
  where t' = 0.1*(t/max(t)) + fixed_table. Pair-summed 64-wide f32 tables are
  precomputed on-chip (tile-managed preproc), stored to DRAM scratch, then
  gathered with InstDMAGatherAnt (gpsimd.dma_gather): one instruction per
  8192-token chunk per table (8 total/core) instead of per-column indirect
  DMAs — Q7 descriptor-gen drops from ~1.1us/32KB to 0.34ns/row.

  Indices are pre-permuted on host into the int16 [16-partition-wrapped,
  replicated x8] layout the gather ucode walks: flat gather slot i reads
  idxs[i%16, i//16] and writes out[i%128, i//128, :]. We order the flat list
  so gather output partition p, column j holds token p*64+j of the chunk,
  making the final store a contiguous per-partition DMA.
"""

from contextlib import ExitStack

import numpy as np

import concourse.bacc as bacc
import concourse.bass_isa as bass_isa
import concourse.mybir as mybir
import concourse.tile as tile
from concourse.bass_utils import run_bass_kernel_spmd

N_CORES = 8
B, M, R, D = 16, 512, 32, 128
TABLE_LEN = 4106
T = (B // N_CORES) * M * R  # 32768 tokens per core
PAIRS = D // 2  # 64
FLAT_N = TABLE_LEN * D // 128  # 4106
PAIR_N = FLAT_N // 2  # 2053
CHUNK = 4096
NCHUNK = T // CHUNK  # 8
C = CHUNK // 128  # 32 tokens per partition per chunk
IDX_COLS = CHUNK // 16  # 256 idx columns per chunk

F32 = mybir.dt.float32
I16 = mybir.dt.int16


def _flat(h, p):
    return h[:].rearrange("a b -> (a b)").rearrange("(p n) -> p n", p=p)


def build_nc():
    nc = bacc.Bacc("TRN2", target_bir_lowering=False, debug=False)
    idx_x = nc.dram_tensor("idx_x", [128, NCHUNK * IDX_COLS], I16, kind="ExternalInput")
    idx_y = nc.dram_tensor("idx_y", [128, NCHUNK * IDX_COLS], I16, kind="ExternalInput")
    fixed = nc.dram_tensor("fixed_table", [TABLE_LEN, D], F32, kind="ExternalInput")
    tx = nc.dram_tensor("table_x", [TABLE_LEN, D], F32, kind="ExternalInput")
    ty = nc.dram_tensor("table_y", [TABLE_LEN, D], F32, kind="ExternalInput")
    out = nc.dram_tensor("out", [T, D], F32, kind="ExternalOutput")
    txp_d = nc.dram_tensor("txp", [TABLE_LEN, PAIRS], F32, kind="Internal")
    typ_d = nc.dram_tensor("typ", [TABLE_LEN, PAIRS], F32, kind="Internal")

    with tile.TileContext(nc) as tc, ExitStack() as ctx:
        # x table first on the sync HWDGE ring; fixed + y on the scalar ring
        # (parallel) -- x's critical path to the first gather never waits on
        # y. idx tiles load behind xt on sync (needed only once x scratch is
        # stored).
        prep = ctx.enter_context(tc.tile_pool(name="prep", bufs=1))
        ip = ctx.enter_context(tc.tile_pool(name="idx", bufs=1))
        xt = prep.tile([128, FLAT_N], F32, tag="xt", name="xt")
        yt = prep.tile([128, FLAT_N], F32, tag="yt", name="yt")
        ft = prep.tile([128, FLAT_N], F32, tag="ft", name="ft")
        ix = ip.tile([128, NCHUNK * IDX_COLS], I16, tag="ix", name="ix")
        iy = ip.tile([128, NCHUNK * IDX_COLS], I16, tag="iy", name="iy")
        nc.sync.dma_start(xt[:], _flat(tx, 128))
        nc.sync.dma_start(ix[:], idx_x[:])
        nc.sync.dma_start(iy[:], idx_y[:])
        nc.scalar.dma_start(ft[:], _flat(fixed, 128))
        nc.scalar.dma_start(yt[:], _flat(ty, 128))

        # x's reduce_max is the first DVE op so it runs as soon as xt lands,
        # ahead of the fixed-table pair-sum (which waits on ft anyway).
        mxx = prep.tile([128, 1], F32, tag="mxx", name="mxx")
        nc.vector.reduce_max(mxx[:], xt[:], axis=mybir.AxisListType.X)

        fp = prep.tile([128, PAIR_N], F32, tag="fp", name="fp")
        fr = ft[:].rearrange("p (n two) -> p n two", two=2)
        nc.vector.tensor_add(fp[:], fr[:, :, 0], fr[:, :, 1])

        # per-table scratch build + DRAM RAW fence, all tile-tracked:
        #   store reads ps  ->  readback WRITES ps[:, 0:4] (WAR: waits for the
        #   store's completion sem, i.e. bytes landed in DRAM)  ->  per-chunk
        #   DVE copy reads ps into the gather's out tile (RAW)  ->  gather
        #   overwrites that tile (WAW). Every gather thus starts only after
        #   its table's scratch store completed; tile cannot reorder it away.
        pss = []
        for nm, src_t, dram in (("x", xt, txp_d), ("y", yt, typ_d)):
            if nm == "x":
                mx = mxx
            else:
                mx = prep.tile([128, 1], F32, tag=f"mx{nm}", name=f"mx{nm}")
                nc.vector.reduce_max(mx[:], src_t[:], axis=mybir.AxisListType.X)
            gm = prep.tile([128, 1], F32, tag=f"gm{nm}", name=f"gm{nm}")
            nc.gpsimd.partition_all_reduce(gm[:], mx[:], 128, bass_isa.ReduceOp.max)
            sc = prep.tile([128, 1], F32, tag=f"sc{nm}", name=f"sc{nm}")
            nc.vector.reciprocal(sc[:], gm[:])
            nc.vector.tensor_scalar_mul(sc[:], sc[:], 0.1)
            pr = src_t[:].rearrange("p (n two) -> p n two", two=2)
            ps = prep.tile([128, PAIR_N], F32, tag=f"ps{nm}", name=f"ps{nm}")
            nc.vector.tensor_add(ps[:], pr[:, :, 0], pr[:, :, 1])
            nc.vector.scalar_tensor_tensor(
                ps[:], ps[:], sc[:, 0:1], fp[:],
                op0=mybir.AluOpType.mult, op1=mybir.AluOpType.add,
            )
            nc.sync.dma_start(_flat(dram, 128), ps[:])
            nc.sync.dma_start(ps[:, 0:4], _flat(dram, 128)[:, 0:4])
            pss.append(ps)

        # ---- main loop: per chunk, one dma_gather per table, merge, store
        gp = ctx.enter_context(tc.tile_pool(name="g", bufs=3))
        mp = ctx.enter_context(tc.tile_pool(name="m", bufs=3))

        def gather(tab_i, k, dst):
            src = txp_d if tab_i == 0 else typ_d
            idxs = ix if tab_i == 0 else iy
            nc.vector.tensor_copy(dst[:, 0, 0:4], pss[tab_i][:, 0:4])
            nc.gpsimd.dma_gather(
                dst[:], src[:], idxs[:, k * IDX_COLS : (k + 1) * IDX_COLS],
                CHUNK, CHUNK, PAIRS, single_packet=False,
            )

        for k in range(NCHUNK):
            gx = gp.tile([128, C, PAIRS], F32, tag="gx")
            gy = gp.tile([128, C, PAIRS], F32, tag="gy")
            gather(0, k, gx)
            gather(1, k, gy)
            mg = mp.tile([128, C, D], F32, tag="mg")
            nc.vector.tensor_copy(mg[:, :, 0:PAIRS], gx[:])
            nc.vector.tensor_copy(mg[:, :, PAIRS:D], gy[:])
            nc.sync.dma_start(
                out[k * CHUNK : (k + 1) * CHUNK, :].rearrange(
                    "(p c) f -> p (c f)", p=128
                ),
                mg[:].rearrange("p c f -> p (c f)"),
            )

    nc.compile()
    return nc


# host-side index marshalling: flat gather slot i of chunk k must read token
# (i%128)*64 + i//128 so the gather output lands partition-major; the int16
# tile stores slot i at [i%16, i//16], replicated across the 8 Q7 cores.
_slot2tok = None


def _make_idx(idx_col):
    global _slot2tok
    if _slot2tok is None:
        i = np.arange(CHUNK)
        _slot2tok = (i % 128) * C + i // 128
    blocks = []
    for k in range(NCHUNK):
        L = idx_col[k * CHUNK + _slot2tok]
        blocks.append(L.reshape(IDX_COLS, 16).T)
    return np.ascontiguousarray(np.tile(np.concatenate(blocks, axis=1), (8, 1)))


def make_in_maps(positions, fixed_table, table_x, table_y):
    pos_flat = positions.reshape(-1, 2)
    idx = np.where(pos_flat < 0, 1, pos_flat).astype(np.int16)
    fixed_table = np.ascontiguousarray(fixed_table, dtype=np.float32)
    table_x = np.ascontiguousarray(table_x, dtype=np.float32)
    table_y = np.ascontiguousarray(table_y, dtype=np.float32)
    in_maps = []
    for c in range(N_CORES):
        s = idx[c * T : (c + 1) * T]
        in_maps.append(
            {
                "idx_x": _make_idx(s[:, 0]),
                "idx_y": _make_idx(s[:, 1]),
                "fixed_table": fixed_table,
                "table_x": table_x,
                "table_y": table_y,
            }
        )
    return in_maps


_cache = {}


def kernel(positions, fixed_table, table_x, table_y):
    nc = _cache.get("nc")
    if nc is None:
        nc = _cache["nc"] = build_nc()
    in_maps = make_in_maps(positions, fixed_table, table_x, table_y)
    res = run_bass_kernel_spmd(nc, in_maps, core_ids=list(range(N_CORES)))
    outs = [r["out"] for r in res.results]
    return np.concatenate(outs, axis=0).reshape(B, M, R, D)


# revision 11
# speedup vs baseline: 1.1646x; 1.1646x over previous
"""Trainium2 Bass kernel for nn_LookupTableLayer (embedding_lookup).

Full-input contract: kernel(**inputs) takes the full unsharded numpy inputs,
shards positions across 8 NeuronCores (batch dim), runs one SPMD NEFF on
cores 0-7, and returns the full [16, 512, 32, 128] f32 output.

Algorithm:
  out[..., 0:64] = pairsum(tx')[idx0], out[..., 64:128] = pairsum(ty')[idx1]
  where t' = 0.1*(t/max(t)) + fixed_table. Pair-summed 64-wide f32 tables are
  precomputed on-chip (tile-managed preproc), stored to DRAM scratch, then
  gathered with InstDMAGatherAnt (gpsimd.dma_gather): one instruction per
  8192-token chunk per table (8 total/core) instead of per-column indirect
  DMAs — Q7 descriptor-gen drops from ~1.1us/32KB to 0.34ns/row.

  Indices are pre-permuted on host into the int16 [16-partition-wrapped,
  replicated x8] layout the gather ucode walks: flat gather slot i reads
  idxs[i%16, i//16] and writes out[i%128, i//128, :]. We order the flat list
  so gather output partition p, column j holds token p*64+j of the chunk,
  making the final store a contiguous per-partition DMA.
"""

from contextlib import ExitStack

import numpy as np

import concourse.bacc as bacc
import concourse.bass_isa as bass_isa
import concourse.mybir as mybir
import concourse.tile as tile
from concourse.bass_utils import run_bass_kernel_spmd

N_CORES = 8
B, M, R, D = 16, 512, 32, 128
TABLE_LEN = 4106
T = (B // N_CORES) * M * R  # 32768 tokens per core
PAIRS = D // 2  # 64
FLAT_N = TABLE_LEN * D // 128  # 4106
PAIR_N = FLAT_N // 2  # 2053
CHUNK = 4096
NCHUNK = T // CHUNK  # 8
C = CHUNK // 128  # 32 tokens per partition per chunk
IDX_COLS = CHUNK // 16  # 256 idx columns per chunk

F32 = mybir.dt.float32
I16 = mybir.dt.int16


def _flat(h, p):
    return h[:].rearrange("a b -> (a b)").rearrange("(p n) -> p n", p=p)


def build_nc():
    nc = bacc.Bacc("TRN2", target_bir_lowering=False, debug=False)
    idx_x = nc.dram_tensor("idx_x", [128, NCHUNK * IDX_COLS], I16, kind="ExternalInput")
    idx_y = nc.dram_tensor("idx_y", [128, NCHUNK * IDX_COLS], I16, kind="ExternalInput")
    fixed = nc.dram_tensor("fixed_table", [TABLE_LEN, D], F32, kind="ExternalInput")
    tx = nc.dram_tensor("table_x", [TABLE_LEN, D], F32, kind="ExternalInput")
    ty = nc.dram_tensor("table_y", [TABLE_LEN, D], F32, kind="ExternalInput")
    out = nc.dram_tensor("out", [T, D], F32, kind="ExternalOutput")
    txp_d = nc.dram_tensor("txp", [TABLE_LEN, PAIRS], F32, kind="Internal")
    typ_d = nc.dram_tensor("typ", [TABLE_LEN, PAIRS], F32, kind="Internal")

    with tile.TileContext(nc) as tc, ExitStack() as ctx:
        # x table first on the sync HWDGE ring; fixed + y on the scalar ring
        # (parallel) -- x's critical path to the first gather never waits on
        # y. idx tiles load behind xt on sync (needed only once x scratch is
        # stored).
        prep = ctx.enter_context(tc.tile_pool(name="prep", bufs=1))
        ip = ctx.enter_context(tc.tile_pool(name="idx", bufs=1))
        xt = prep.tile([128, FLAT_N], F32, tag="xt", name="xt")
        yt = prep.tile([128, FLAT_N], F32, tag="yt", name="yt")
        ft = prep.tile([128, FLAT_N], F32, tag="ft", name="ft")
        ix = ip.tile([128, NCHUNK * IDX_COLS], I16, tag="ix", name="ix")
        iy = ip.tile([128, NCHUNK * IDX_COLS], I16, tag="iy", name="iy")
        nc.sync.dma_start(xt[:], _flat(tx, 128))
        nc.sync.dma_start(ix[:], idx_x[:])
        nc.sync.dma_start(iy[:], idx_y[:])
        nc.scalar.dma_start(ft[:], _flat(fixed, 128))
        nc.scalar.dma_start(yt[:], _flat(ty, 128))

        fp = prep.tile([128, PAIR_N], F32, tag="fp", name="fp")
        fr = ft[:].rearrange("p (n two) -> p n two", two=2)
        nc.vector.tensor_add(fp[:], fr[:, :, 0], fr[:, :, 1])

        # per-table scratch build + DRAM RAW fence, all tile-tracked:
        #   store reads ps  ->  readback WRITES ps[:, 0:4] (WAR: waits for the
        #   store's completion sem, i.e. bytes landed in DRAM)  ->  per-chunk
        #   DVE copy reads ps into the gather's out tile (RAW)  ->  gather
        #   overwrites that tile (WAW). Every gather thus starts only after
        #   its table's scratch store completed; tile cannot reorder it away.
        pss = []
        for nm, src_t, dram in (("x", xt, txp_d), ("y", yt, typ_d)):
            mx = prep.tile([128, 1], F32, tag=f"mx{nm}", name=f"mx{nm}")
            nc.vector.reduce_max(mx[:], src_t[:], axis=mybir.AxisListType.X)
            gm = prep.tile([128, 1], F32, tag=f"gm{nm}", name=f"gm{nm}")
            nc.gpsimd.partition_all_reduce(gm[:], mx[:], 128, bass_isa.ReduceOp.max)
            sc = prep.tile([128, 1], F32, tag=f"sc{nm}", name=f"sc{nm}")
            nc.vector.reciprocal(sc[:], gm[:])
            nc.vector.tensor_scalar_mul(sc[:], sc[:], 0.1)
            pr = src_t[:].rearrange("p (n two) -> p n two", two=2)
            ps = prep.tile([128, PAIR_N], F32, tag=f"ps{nm}", name=f"ps{nm}")
            nc.vector.tensor_add(ps[:], pr[:, :, 0], pr[:, :, 1])
            nc.vector.scalar_tensor_tensor(
                ps[:], ps[:], sc[:, 0:1], fp[:],
                op0=mybir.AluOpType.mult, op1=mybir.AluOpType.add,
            )
            nc.sync.dma_start(_flat(dram, 128), ps[:])
            nc.sync.dma_start(ps[:, 0:4], _flat(dram, 128)[:, 0:4])
            pss.append(ps)

        # ---- main loop: per chunk, one dma_gather per table, merge, store
        gp = ctx.enter_context(tc.tile_pool(name="g", bufs=3))
        mp = ctx.enter_context(tc.tile_pool(name="m", bufs=3))

        def gather(tab_i, k, dst):
            src = txp_d if tab_i == 0 else typ_d
            idxs = ix if tab_i == 0 else iy
            nc.vector.tensor_copy(dst[:, 0, 0:4], pss[tab_i][:, 0:4])
            nc.gpsimd.dma_gather(
                dst[:], src[:], idxs[:, k * IDX_COLS : (k + 1) * IDX_COLS],
                CHUNK, CHUNK, PAIRS, single_packet=False,
            )

        for k in range(NCHUNK):
            gx = gp.tile([128, C, PAIRS], F32, tag="gx")
            gy = gp.tile([128, C, PAIRS], F32, tag="gy")
            gather(0, k, gx)
            gather(1, k, gy)
            mg = mp.tile([128, C, D], F32, tag="mg")
            nc.vector.tensor_copy(mg[:, :, 0:PAIRS], gx[:])
            nc.vector.tensor_copy(mg[:, :, PAIRS:D], gy[:])
            nc.sync.dma_start(
                out[k * CHUNK : (k + 1) * CHUNK, :].rearrange(
                    "(p c) f -> p (c f)", p=128
                ),
                mg[:].rearrange("p c f -> p (c f)"),
            )

    nc.compile()
    return nc


# host-side index marshalling: flat gather slot i of chunk k must read token
# (i%128)*64 + i//128 so the gather output lands partition-major; the int16
# tile stores slot i at [i%16, i//16], replicated across the 8 Q7 cores.
_slot2tok = None


def _make_idx(idx_col):
    global _slot2tok
    if _slot2tok is None:
        i = np.arange(CHUNK)
        _slot2tok = (i % 128) * C + i // 128
    blocks = []
    for k in range(NCHUNK):
        L = idx_col[k * CHUNK + _slot2tok]
        blocks.append(L.reshape(IDX_COLS, 16).T)
    return np.ascontiguousarray(np.tile(np.concatenate(blocks, axis=1), (8, 1)))


def make_in_maps(positions, fixed_table, table_x, table_y):
    pos_flat = positions.reshape(-1, 2)
    idx = np.where(pos_flat < 0, 1, pos_flat).astype(np.int16)
    fixed_table = np.ascontiguousarray(fixed_table, dtype=np.float32)
    table_x = np.ascontiguousarray(table_x, dtype=np.float32)
    table_y = np.ascontiguousarray(table_y, dtype=np.float32)
    in_maps = []
    for c in range(N_CORES):
        s = idx[c * T : (c + 1) * T]
        in_maps.append(
            {
                "idx_x": _make_idx(s[:, 0]),
                "idx_y": _make_idx(s[:, 1]),
                "fixed_table": fixed_table,
                "table_x": table_x,
                "table_y": table_y,
            }
        )
    return in_maps


_cache = {}


def kernel(positions, fixed_table, table_x, table_y):
    nc = _cache.get("nc")
    if nc is None:
        nc = _cache["nc"] = build_nc()
    in_maps = make_in_maps(positions, fixed_table, table_x, table_y)
    res = run_bass_kernel_spmd(nc, in_maps, core_ids=list(range(N_CORES)))
    outs = [r["out"] for r in res.results]
    return np.concatenate(outs, axis=0).reshape(B, M, R, D)


# revision 13
# speedup vs baseline: 1.1657x; 1.0010x over previous
"""Trainium2 Bass kernel for nn_LookupTableLayer (embedding_lookup).

Full-input contract: kernel(**inputs) takes the full unsharded numpy inputs,
shards positions across 8 NeuronCores (batch dim), runs one SPMD NEFF on
cores 0-7, and returns the full [16, 512, 32, 128] f32 output.

Algorithm:
  out[..., 0:64] = pairsum(tx')[idx0], out[..., 64:128] = pairsum(ty')[idx1]
  where t' = 0.1*(t/max(t)) + fixed_table. Pair-summed 64-wide f32 tables are
  precomputed on-chip (tile-managed preproc), stored to DRAM scratch, then
  gathered with InstDMAGatherAnt (gpsimd.dma_gather): one instruction per
  4096-token chunk per table (16 total/core, single_packet=False — packed
  mode is capped at 64 descs/lane = 1024 idxs) instead of 512 per-column
  indirect DMAs. Q7 descriptor-gen runs at ~8ns/idx either way and is the
  kernel's floor (~520us/core); 4096-idx chunks keep the ring (1024
  descs/lane) triple-buffered so desc-gen never stalls on reclaim.

  Indices are pre-permuted on host into the int16 [16-partition-wrapped,
  replicated x8] layout the gather ucode walks: flat gather slot i reads
  idxs[i%16, i//16] and writes out[i%128, i//128, :]. We order the flat list
  so gather output partition p, column j holds token p*C+j of the chunk,
  making the final store a contiguous per-partition DMA.
"""

from contextlib import ExitStack

import numpy as np

import concourse.bacc as bacc
import concourse.bass_isa as bass_isa
import concourse.mybir as mybir
import concourse.tile as tile
from concourse.bass_utils import run_bass_kernel_spmd

N_CORES = 8
B, M, R, D = 16, 512, 32, 128
TABLE_LEN = 4106
T = (B // N_CORES) * M * R  # 32768 tokens per core
PAIRS = D // 2  # 64
FLAT_N = TABLE_LEN * D // 128  # 4106
PAIR_N = FLAT_N // 2  # 2053
CHUNK = 4096
NCHUNK = T // CHUNK  # 8
C = CHUNK // 128  # 32 tokens per partition per chunk
IDX_COLS = CHUNK // 16  # 256 idx columns per chunk

F32 = mybir.dt.float32
I16 = mybir.dt.int16


def _flat(h, p):
    return h[:].rearrange("a b -> (a b)").rearrange("(p n) -> p n", p=p)


def build_nc():
    nc = bacc.Bacc("TRN2", target_bir_lowering=False, debug=False)
    idx_x = nc.dram_tensor("idx_x", [128, NCHUNK * IDX_COLS], I16, kind="ExternalInput")
    idx_y = nc.dram_tensor("idx_y", [128, NCHUNK * IDX_COLS], I16, kind="ExternalInput")
    fixed = nc.dram_tensor("fixed_table", [TABLE_LEN, D], F32, kind="ExternalInput")
    tx = nc.dram_tensor("table_x", [TABLE_LEN, D], F32, kind="ExternalInput")
    ty = nc.dram_tensor("table_y", [TABLE_LEN, D], F32, kind="ExternalInput")
    out = nc.dram_tensor("out", [T, D], F32, kind="ExternalOutput")
    txp_d = nc.dram_tensor("txp", [TABLE_LEN, PAIRS], F32, kind="Internal")
    typ_d = nc.dram_tensor("typ", [TABLE_LEN, PAIRS], F32, kind="Internal")

    with tile.TileContext(nc) as tc, ExitStack() as ctx:
        # x table first on the sync HWDGE ring; fixed + y on the scalar ring
        # (parallel) -- x's critical path to the first gather never waits on
        # y. idx tiles load behind xt on sync (needed only once x scratch is
        # stored).
        prep = ctx.enter_context(tc.tile_pool(name="prep", bufs=1))
        ip = ctx.enter_context(tc.tile_pool(name="idx", bufs=1))
        xt = prep.tile([128, FLAT_N], F32, tag="xt", name="xt")
        yt = prep.tile([128, FLAT_N], F32, tag="yt", name="yt")
        ft = prep.tile([128, FLAT_N], F32, tag="ft", name="ft")
        ix = ip.tile([128, NCHUNK * IDX_COLS], I16, tag="ix", name="ix")
        iy = ip.tile([128, NCHUNK * IDX_COLS], I16, tag="iy", name="iy")
        nc.sync.dma_start(xt[:], _flat(tx, 128))
        nc.sync.dma_start(ix[:], idx_x[:])
        nc.sync.dma_start(iy[:], idx_y[:])
        nc.scalar.dma_start(ft[:], _flat(fixed, 128))
        nc.scalar.dma_start(yt[:], _flat(ty, 128))

        fp = prep.tile([128, PAIR_N], F32, tag="fp", name="fp")
        fr = ft[:].rearrange("p (n two) -> p n two", two=2)
        nc.vector.tensor_add(fp[:], fr[:, :, 0], fr[:, :, 1])

        # per-table scratch build + DRAM RAW fence, all tile-tracked:
        #   store reads ps  ->  readback WRITES ps[:, 0:4] (WAR: waits for the
        #   store's completion sem, i.e. bytes landed in DRAM)  ->  per-chunk
        #   DVE copy reads ps into the gather's out tile (RAW)  ->  gather
        #   overwrites that tile (WAW). Every gather thus starts only after
        #   its table's scratch store completed; tile cannot reorder it away.
        pss = []
        for nm, src_t, dram in (("x", xt, txp_d), ("y", yt, typ_d)):
            mx = prep.tile([128, 1], F32, tag=f"mx{nm}", name=f"mx{nm}")
            nc.vector.reduce_max(mx[:], src_t[:], axis=mybir.AxisListType.X)
            gm = prep.tile([128, 1], F32, tag=f"gm{nm}", name=f"gm{nm}")
            nc.gpsimd.partition_all_reduce(gm[:], mx[:], 128, bass_isa.ReduceOp.max)
            sc = prep.tile([128, 1], F32, tag=f"sc{nm}", name=f"sc{nm}")
            nc.vector.reciprocal(sc[:], gm[:])
            nc.vector.tensor_scalar_mul(sc[:], sc[:], 0.1)
            pr = src_t[:].rearrange("p (n two) -> p n two", two=2)
            ps = prep.tile([128, PAIR_N], F32, tag=f"ps{nm}", name=f"ps{nm}")
            nc.vector.tensor_add(ps[:], pr[:, :, 0], pr[:, :, 1])
            nc.vector.scalar_tensor_tensor(
                ps[:], ps[:], sc[:, 0:1], fp[:],
                op0=mybir.AluOpType.mult, op1=mybir.AluOpType.add,
            )
            nc.sync.dma_start(_flat(dram, 128), ps[:])
            nc.sync.dma_start(ps[:, 0:4], _flat(dram, 128)[:, 0:4])
            pss.append(ps)

        # ---- main loop: per chunk, one dma_gather per table, merge, store
        gp = ctx.enter_context(tc.tile_pool(name="g", bufs=3))
        mp = ctx.enter_context(tc.tile_pool(name="m", bufs=3))

        def gather(tab_i, k, dst):
            src = txp_d if tab_i == 0 else typ_d
            idxs = ix if tab_i == 0 else iy
            nc.vector.tensor_copy(dst[:, 0, 0:4], pss[tab_i][:, 0:4])
            nc.gpsimd.dma_gather(
                dst[:], src[:], idxs[:, k * IDX_COLS : (k + 1) * IDX_COLS],
                CHUNK, CHUNK, PAIRS, single_packet=False,
            )

        for k in range(NCHUNK):
            gx = gp.tile([128, C, PAIRS], F32, tag="gx")
            gy = gp.tile([128, C, PAIRS], F32, tag="gy")
            gather(0, k, gx)
            gather(1, k, gy)
            mg = mp.tile([128, C, D], F32, tag="mg")
            nc.vector.tensor_copy(mg[:, :, 0:PAIRS], gx[:])
            nc.vector.tensor_copy(mg[:, :, PAIRS:D], gy[:])
            nc.sync.dma_start(
                out[k * CHUNK : (k + 1) * CHUNK, :].rearrange(
                    "(p c) f -> p (c f)", p=128
                ),
                mg[:].rearrange("p c f -> p (c f)"),
            )

    nc.compile()
    return nc


# host-side index marshalling: flat gather slot i of chunk k must read token
# (i%128)*C + i//128 so the gather output lands partition-major; the int16
# tile stores slot i at [i%16, i//16], replicated across the 8 Q7 cores.
_slot2tok = None


def _make_idx(idx_col):
    global _slot2tok
    if _slot2tok is None:
        i = np.arange(CHUNK)
        _slot2tok = (i % 128) * C + i // 128
    blocks = []
    for k in range(NCHUNK):
        L = idx_col[k * CHUNK + _slot2tok]
        blocks.append(L.reshape(IDX_COLS, 16).T)
    return np.ascontiguousarray(np.tile(np.concatenate(blocks, axis=1), (8, 1)))


def make_in_maps(positions, fixed_table, table_x, table_y):
    pos_flat = positions.reshape(-1, 2)
    idx = np.where(pos_flat < 0, 1, pos_flat).astype(np.int16)
    fixed_table = np.ascontiguousarray(fixed_table, dtype=np.float32)
    table_x = np.ascontiguousarray(table_x, dtype=np.float32)
    table_y = np.ascontiguousarray(table_y, dtype=np.float32)
    in_maps = []
    for c in range(N_CORES):
        s = idx[c * T : (c + 1) * T]
        in_maps.append(
            {
                "idx_x": _make_idx(s[:, 0]),
                "idx_y": _make_idx(s[:, 1]),
                "fixed_table": fixed_table,
                "table_x": table_x,
                "table_y": table_y,
            }
        )
    return in_maps


_cache = {}


def kernel(positions, fixed_table, table_x, table_y):
    nc = _cache.get("nc")
    if nc is None:
        nc = _cache["nc"] = build_nc()
    in_maps = make_in_maps(positions, fixed_table, table_x, table_y)
    res = run_bass_kernel_spmd(nc, in_maps, core_ids=list(range(N_CORES)))
    outs = [r["out"] for r in res.results]
    return np.concatenate(outs, axis=0).reshape(B, M, R, D)


# revision 16
# speedup vs baseline: 1.1682x; 1.0021x over previous
"""Trainium2 Bass kernel for nn_LookupTableLayer (embedding_lookup).

Full-input contract: kernel(**inputs) takes the full unsharded numpy inputs,
shards positions across 8 NeuronCores (batch dim), runs one SPMD NEFF on
cores 0-7, and returns the full [16, 512, 32, 128] f32 output.

Algorithm:
  out[..., 0:64] = pairsum(tx')[idx0], out[..., 64:128] = pairsum(ty')[idx1]
  where t' = 0.1*(t/max(t)) + fixed_table. Pair-summed 64-wide f32 tables are
  precomputed on-chip (tile-managed preproc), stored to DRAM scratch, then
  gathered with InstDMAGatherAnt (gpsimd.dma_gather): one instruction per
  4096-token chunk per table (16 total/core, single_packet=False — packed
  mode is capped at 64 descs/lane = 1024 idxs) instead of 512 per-column
  indirect DMAs. Q7 descriptor-gen runs at ~8ns/idx either way and is the
  kernel's floor (~520us/core); 4096-idx chunks keep the ring (1024
  descs/lane) triple-buffered so desc-gen never stalls on reclaim.

  Indices are pre-permuted on host into the int16 [16-partition-wrapped,
  replicated x8] layout the gather ucode walks: flat gather slot i reads
  idxs[i%16, i//16] and writes out[i%128, i//128, :]. We order the flat list
  so gather output partition p, column j holds token p*C+j of the chunk,
  making the final store a contiguous per-partition DMA.
"""

from contextlib import ExitStack

import numpy as np

import concourse.bacc as bacc
import concourse.bass_isa as bass_isa
import concourse.mybir as mybir
import concourse.tile as tile
from concourse.bass_utils import run_bass_kernel_spmd

N_CORES = 8
B, M, R, D = 16, 512, 32, 128
TABLE_LEN = 4106
T = (B // N_CORES) * M * R  # 32768 tokens per core
PAIRS = D // 2  # 64
FLAT_N = TABLE_LEN * D // 128  # 4106
PAIR_N = FLAT_N // 2  # 2053
CHUNK = 4096
NCHUNK = T // CHUNK  # 8
C = CHUNK // 128  # 32 tokens per partition per chunk
IDX_COLS = CHUNK // 16  # 256 idx columns per chunk

F32 = mybir.dt.float32
I16 = mybir.dt.int16


def _flat(h, p):
    return h[:].rearrange("a b -> (a b)").rearrange("(p n) -> p n", p=p)


def build_nc():
    nc = bacc.Bacc("TRN2", target_bir_lowering=False, debug=False)
    idx_x = nc.dram_tensor("idx_x", [128, NCHUNK * IDX_COLS], I16, kind="ExternalInput")
    idx_y = nc.dram_tensor("idx_y", [128, NCHUNK * IDX_COLS], I16, kind="ExternalInput")
    fixed = nc.dram_tensor("fixed_table", [TABLE_LEN, D], F32, kind="ExternalInput")
    tx = nc.dram_tensor("table_x", [TABLE_LEN, D], F32, kind="ExternalInput")
    ty = nc.dram_tensor("table_y", [TABLE_LEN, D], F32, kind="ExternalInput")
    out = nc.dram_tensor("out", [T, D], F32, kind="ExternalOutput")
    txp_d = nc.dram_tensor("txp", [TABLE_LEN, PAIRS], F32, kind="Internal")
    typ_d = nc.dram_tensor("typ", [TABLE_LEN, PAIRS], F32, kind="Internal")

    with tile.TileContext(nc) as tc, ExitStack() as ctx:
        # x table first on the sync HWDGE ring; fixed + y on the scalar ring
        # (parallel) -- x's critical path to the first gather never waits on
        # y. idx tiles load behind xt on sync (needed only once x scratch is
        # stored).
        prep = ctx.enter_context(tc.tile_pool(name="prep", bufs=1))
        ip = ctx.enter_context(tc.tile_pool(name="idx", bufs=1))
        xt = prep.tile([128, FLAT_N], F32, tag="xt", name="xt")
        yt = prep.tile([128, FLAT_N], F32, tag="yt", name="yt")
        ft = prep.tile([128, FLAT_N], F32, tag="ft", name="ft")
        ix = ip.tile([128, NCHUNK * IDX_COLS], I16, tag="ix", name="ix")
        iy = ip.tile([128, NCHUNK * IDX_COLS], I16, tag="iy", name="iy")
        nc.sync.dma_start(xt[:], _flat(tx, 128))
        nc.sync.dma_start(ix[:], idx_x[:])
        nc.sync.dma_start(iy[:], idx_y[:])
        nc.scalar.dma_start(ft[:], _flat(fixed, 128))
        nc.scalar.dma_start(yt[:], _flat(ty, 128))

        fp = prep.tile([128, PAIR_N], F32, tag="fp", name="fp")
        fr = ft[:].rearrange("p (n two) -> p n two", two=2)
        nc.vector.tensor_add(fp[:], fr[:, :, 0], fr[:, :, 1])

        # per-table scratch build + DRAM RAW fence, all tile-tracked:
        #   store reads ps  ->  readback WRITES ps[:, 0:4] (WAR: waits for the
        #   store's completion sem, i.e. bytes landed in DRAM)  ->  per-chunk
        #   DVE copy reads ps into the gather's out tile (RAW)  ->  gather
        #   overwrites that tile (WAW). Every gather thus starts only after
        #   its table's scratch store completed; tile cannot reorder it away.
        pss = []
        for nm, src_t, dram in (("x", xt, txp_d), ("y", yt, typ_d)):
            mx = prep.tile([128, 1], F32, tag=f"mx{nm}", name=f"mx{nm}")
            nc.vector.reduce_max(mx[:], src_t[:], axis=mybir.AxisListType.X)
            gm = prep.tile([128, 1], F32, tag=f"gm{nm}", name=f"gm{nm}")
            nc.gpsimd.partition_all_reduce(gm[:], mx[:], 128, bass_isa.ReduceOp.max)
            sc = prep.tile([128, 1], F32, tag=f"sc{nm}", name=f"sc{nm}")
            nc.vector.reciprocal(sc[:], gm[:])
            nc.vector.tensor_scalar_mul(sc[:], sc[:], 0.1)
            pr = src_t[:].rearrange("p (n two) -> p n two", two=2)
            ps = prep.tile([128, PAIR_N], F32, tag=f"ps{nm}", name=f"ps{nm}")
            nc.vector.tensor_add(ps[:], pr[:, :, 0], pr[:, :, 1])
            nc.vector.scalar_tensor_tensor(
                ps[:], ps[:], sc[:, 0:1], fp[:],
                op0=mybir.AluOpType.mult, op1=mybir.AluOpType.add,
            )
            nc.sync.dma_start(_flat(dram, 128), ps[:])
            nc.sync.dma_start(ps[:, 0:4], _flat(dram, 128)[:, 0:4])
            pss.append(ps)

        # ---- main loop: per chunk, one dma_gather per table, merge, store
        gp = ctx.enter_context(tc.tile_pool(name="g", bufs=3))
        mp = ctx.enter_context(tc.tile_pool(name="m", bufs=3))

        def gather(tab_i, k, dst):
            src = txp_d if tab_i == 0 else typ_d
            idxs = ix if tab_i == 0 else iy
            nc.vector.tensor_copy(dst[:, 0, 0:4], pss[tab_i][:, 0:4])
            nc.gpsimd.dma_gather(
                dst[:], src[:], idxs[:, k * IDX_COLS : (k + 1) * IDX_COLS],
                CHUNK, CHUNK, PAIRS, single_packet=False,
            )

        for k in range(NCHUNK):
            gx = gp.tile([128, C, PAIRS], F32, tag="gx")
            gy = gp.tile([128, C, PAIRS], F32, tag="gy")
            gather(0, k, gx)
            gather(1, k, gy)
            mg = mp.tile([128, C, D], F32, tag="mg")
            nc.vector.tensor_copy(mg[:, :, 0:PAIRS], gx[:])
            nc.vector.tensor_copy(mg[:, :, PAIRS:D], gy[:])
            nc.sync.dma_start(
                out[k * CHUNK : (k + 1) * CHUNK, :].rearrange(
                    "(p c) f -> p (c f)", p=128
                ),
                mg[:].rearrange("p c f -> p (c f)"),
            )

    nc.compile()
    return nc


# host-side index marshalling: flat gather slot i of chunk k must read token
# (i%128)*C + i//128 so the gather output lands partition-major; the int16
# tile stores slot i at [i%16, i//16], replicated across the 8 Q7 cores.
_slot2tok = None


def _make_idx(idx_col):
    global _slot2tok
    if _slot2tok is None:
        i = np.arange(CHUNK)
        _slot2tok = (i % 128) * C + i // 128
    blocks = []
    for k in range(NCHUNK):
        L = idx_col[k * CHUNK + _slot2tok]
        blocks.append(L.reshape(IDX_COLS, 16).T)
    return np.ascontiguousarray(np.tile(np.concatenate(blocks, axis=1), (8, 1)))


def make_in_maps(positions, fixed_table, table_x, table_y):
    pos_flat = positions.reshape(-1, 2)
    idx = np.where(pos_flat < 0, 1, pos_flat).astype(np.int16)
    fixed_table = np.ascontiguousarray(fixed_table, dtype=np.float32)
    table_x = np.ascontiguousarray(table_x, dtype=np.float32)
    table_y = np.ascontiguousarray(table_y, dtype=np.float32)
    in_maps = []
    for c in range(N_CORES):
        s = idx[c * T : (c + 1) * T]
        in_maps.append(
            {
                "idx_x": _make_idx(s[:, 0]),
                "idx_y": _make_idx(s[:, 1]),
                "fixed_table": fixed_table,
                "table_x": table_x,
                "table_y": table_y,
            }
        )
    return in_maps


_cache = {}


def kernel(positions, fixed_table, table_x, table_y):
    nc = _cache.get("nc")
    if nc is None:
        nc = _cache["nc"] = build_nc()
    in_maps = make_in_maps(positions, fixed_table, table_x, table_y)
    res = run_bass_kernel_spmd(nc, in_maps, core_ids=list(range(N_CORES)))
    outs = [r["out"] for r in res.results]
    return np.concatenate(outs, axis=0).reshape(B, M, R, D)
